# revision 12
# baseline (speedup 1.0000x reference)
"""Trainium2 Bass kernel for nn_PairwiseConv (gnn_message_passing).

Reference computation, for each edge e=(i,j) of a sparse adjacency:
    pair[b,o,e] = sum_c W[o,c,0]*x[b,c,i] + W[o,c,1]*x[b,c,j] + bias[o]
    y[b,o,n]    = (sum_{e: i_e=n} pair[b,o,e]) / max(deg_j[n],1)
    y[b,127,n]  = deg_j[n]            (counts channel)
where deg_j[n] = #{e: j_e = n}.

Algebraic reformulation (exact):
    y[b,o,n] = S[b,o,n]*recip[n] + (W0^T x)[b,o,n]*c1[n] + bias[o]*c1[n]
    S        = W1^T (x @ AT),  AT[m,n] = #{e: j_e=m, i_e=n}
    recip[n] = 1/max(deg_j[n],1),  c1[n] = deg_i[n]*recip[n]
Key trick: contract x against the count matrix FIRST (P = x @ AT), then
apply the 128x128 conv weights to the much smaller P. The only heavy
matmul is P, done in fp8(e4m3) DoubleRow mode (256-row contraction per
pass) -- counts are small ints (exact in fp8) and the fp8 error on x
only touches the minority S term of the output.

Sharding: 8 cores x 512 dst-node slices, all 4 batches per core.

Device schedule (per core):
 - sync HWDGE queue: AT pair-blocks, small-first so phase P starts early;
   output DMAs at the end.
 - scalar HWDGE queue: tiny consts, x^T front half pair-blocks, then the
   c1-prescaled bf16 x-slice for the W0 pass.
 - gpsimd (software) queue: x^T back half as one big-row DMA (the
   software DGE is descriptor-rate-bound, so one 8KB-row transfer only).
 - PE: dummy fp8 warm-up matmuls during the DMA wait (p-state ramp),
   1-row f32r broadcast of recip, then 64 fp8 DoubleRow phase-P matmuls;
   per batch a bf16 W1 pass on P*recip, a bf16 W0 pass on the x-slice,
   and a 2-row f32r bias/deg pass, all accumulating in one PSUM bank
   that is then copied out as bf16.
"""

import numpy as np
import ml_dtypes

import concourse.bass as bass
import concourse.mybir as mybir
import concourse.tile as tile
from concourse import bacc
from concourse.bass_utils import run_bass_kernel_spmd

B = 4
C = 128   # in channels
O = 128   # out channels incl. counts row (127 real + zero row)
N = 4096
SLICE = 512   # dst nodes per core
NCORES = 8
MC = 32       # 128-row source chunks
KP = MC // 2  # chunk pairs (DoubleRow)
F32 = mybir.dt.float32
F32R = mybir.dt.float32r  # unused: f32r paths failed neuronx compile
BF16 = mybir.dt.bfloat16
F8 = mybir.dt.float8e4
BF16_NP = ml_dtypes.bfloat16
F8_NP = ml_dtypes.float8_e4m3
DR = mybir.MatmulPerfMode.DoubleRow

# pair-blocks per DMA: small first (early PE start), bigger later
AT_BLOCKS = [(0, 2), (2, 4), (4, 6), (6, 8), (8, 12), (12, 16)]
XT_BLOCKS = [(0, 2), (2, 4), (4, 6), (6, 8)]   # front half on scalar
XT_GP = (8, 16)                                # back half on gpsimd


def prep_inputs(x, W, b, idx_i, idx_j):
    x = np.ascontiguousarray(np.asarray(x, np.float32))
    W = np.asarray(W, np.float32)
    bias = np.asarray(b, np.float32)
    ii = np.asarray(idx_i).astype(np.int64)
    jj = np.asarray(idx_j).astype(np.int64)

    # x^T pair-major [p=m%128, (kp, b, t, c)] -- same for all cores
    # xt8[p, k, b, t, c] = x[b, c, (2k+t)*128 + p]
    xt8 = np.ascontiguousarray(
        x.transpose(2, 0, 1)              # [N, B, C]
        .reshape(KP, 2, 128, B, C)        # [k, t, p, b, c]
        .transpose(2, 0, 3, 1, 4)         # [p, k, b, t, c]
        .reshape(128, KP * B, 2, C)
    ).astype(F8_NP)

    # conv weights as lhsT [c, o], o=127 padded with a zero column
    W0T = np.zeros((128, 128), BF16_NP)
    W0T[:, :127] = W[:, :, 0].T.astype(BF16_NP)
    W1T = np.zeros((128, 128), BF16_NP)
    W1T[:, :127] = W[:, :, 1].T.astype(BF16_NP)

    deg_j = np.bincount(jj, minlength=N).astype(np.float32)
    deg_i = np.bincount(ii, minlength=N).astype(np.float32)
    maxdj = np.maximum(deg_j, 1.0)
    recip = (1.0 / maxdj).astype(np.float32)
    c1 = (deg_i / maxdj).astype(np.float32)

    # bias/deg rank-2 pass: lhsT [2, 128] (bias row + counts-channel row)
    blhs = np.zeros((2, 128), np.float32)
    blhs[0, :127] = bias
    blhs[1, 127] = 1.0
    blhs = blhs.astype(BF16_NP)

    in_maps = []
    for s in range(NCORES):
        base = s * SLICE
        sel = (ii >= base) & (ii < base + SLICE)
        atf = np.zeros((N, SLICE), np.float32)
        np.add.at(atf, (jj[sel], ii[sel] - base), 1.0)
        at8 = np.ascontiguousarray(
            atf.reshape(KP, 2, 128, SLICE).transpose(2, 0, 1, 3)
            .reshape(128, KP, 2, SLICE)
        ).astype(F8_NP)

        rs = recip[base:base + SLICE]
        c1s = c1[base:base + SLICE]
        # bf16 x slice pre-scaled by c1 (covers the deg_i*(W0^T x) term)
        xs = np.ascontiguousarray(
            (x[:, :, base:base + SLICE] * c1s[None, None, :])
            .transpose(1, 0, 2).reshape(128, B * SLICE)
        ).astype(BF16_NP)
        brhs = np.stack([c1s, deg_j[base:base + SLICE]]).astype(BF16_NP)

        in_maps.append({
            "XT8": xt8, "AT8": at8, "XS": xs,
            "RROW": rs.reshape(1, SLICE).astype(BF16_NP), "BLHS": blhs, "BRHS": brhs,
            "W0T": W0T, "W1T": W1T,
        })
    return in_maps


def build_program():
    nc = bacc.Bacc("TRN2", target_bir_lowering=False, debug=False,
                   num_devices=NCORES)

    XT8 = nc.dram_tensor("XT8", [128, KP * B, 2, C], F8, kind="ExternalInput")
    AT8 = nc.dram_tensor("AT8", [128, KP, 2, SLICE], F8, kind="ExternalInput")
    XS = nc.dram_tensor("XS", [128, B * SLICE], BF16, kind="ExternalInput")
    RROW = nc.dram_tensor("RROW", [1, SLICE], BF16, kind="ExternalInput")
    BLHS = nc.dram_tensor("BLHS", [2, 128], BF16, kind="ExternalInput")
    BRHS = nc.dram_tensor("BRHS", [2, SLICE], BF16, kind="ExternalInput")
    W0T = nc.dram_tensor("W0T", [128, 128], BF16, kind="ExternalInput")
    W1T = nc.dram_tensor("W1T", [128, 128], BF16, kind="ExternalInput")
    youts = [nc.dram_tensor(f"y{bi}", [O, SLICE], BF16, kind="ExternalOutput")
             for bi in range(B)]

    with tile.TileContext(nc) as tc:
        with (
            tc.tile_pool(name="big", bufs=1) as bigp,
            tc.tile_pool(name="const", bufs=1) as constp,
            tc.tile_pool(name="psb", bufs=2) as psbp,
            tc.tile_pool(name="ost", bufs=2) as ostp,
            tc.tile_pool(name="ps_P", bufs=1, space="PSUM") as ps_P_p,
            tc.tile_pool(name="ps_y", bufs=2, space="PSUM") as ps_y_p,
            tc.tile_pool(name="ps_w", bufs=1, space="PSUM") as ps_w_p,
        ):
            at = bigp.tile([128, KP, 2, SLICE], F8)
            xt = bigp.tile([128, KP * B, 2, C], F8)
            # tiny consts first on scalar (2-3 descriptors each)
            rrow = constp.tile([1, SLICE], BF16)
            nc.scalar.dma_start(rrow[:], RROW[:])
            blhs = constp.tile([2, 128], BF16)
            nc.scalar.dma_start(blhs[:], BLHS[:])
            brhs = constp.tile([2, SLICE], BF16)
            nc.scalar.dma_start(brhs[:], BRHS[:])
            w0t = constp.tile([128, 128], BF16)
            nc.scalar.dma_start(w0t[:], W0T[:])
            w1t = constp.tile([128, 128], BF16)
            nc.scalar.dma_start(w1t[:], W1T[:])
            # main streams
            for lo, hi in AT_BLOCKS:
                nc.sync.dma_start(at[:, lo:hi, :, :], AT8[:, lo:hi, :, :])
            for lo, hi in XT_BLOCKS:
                nc.scalar.dma_start(xt[:, lo * B:hi * B, :, :],
                                    XT8[:, lo * B:hi * B, :, :])
            lo, hi = XT_GP
            nc.gpsimd.dma_start(xt[:, lo * B:hi * B, :, :],
                                XT8[:, lo * B:hi * B, :, :])
            xs = constp.tile([128, B * SLICE], BF16)
            nc.scalar.dma_start(xs[:], XS[:])

            # PE warm-up on memset tiles while DMA streams in (p-state ramp)
            wlhs = constp.tile([128, 2, C], F8)
            nc.vector.memset(wlhs[:], 0.0)
            wrhs = constp.tile([128, 2, SLICE], F8)
            nc.vector.memset(wrhs[:], 0.0)
            ps_w = ps_w_p.tile([128, SLICE], F32, tag="warm", name="ps_warm")
            for _ in range(10):
                nc.tensor.matmul(ps_w[:], wlhs[:], wrhs[:],
                                 start=True, stop=True,
                                 perf_mode=DR, skip_group_check=True)
            # broadcast recip row to all 128 partitions via 1-row matmul
            ones1 = constp.tile([1, 128], BF16)
            nc.vector.memset(ones1[:], 1.0)
            ps_r = ps_w_p.tile([128, SLICE], F32, tag="rbc", name="ps_rbc")
            nc.tensor.matmul(ps_r[:], ones1[:], rrow[:], start=True, stop=True,
                             skip_group_check=True)
            recipf = constp.tile([128, SLICE], BF16)
            nc.vector.tensor_copy(recipf[:], ps_r[:])

            # phase P: ps_P[b] += xT[pair k, b]^T @ AT[pair k] (fp8 DoubleRow)
            ps_Ps = [ps_P_p.tile([128, SLICE], F32, tag=f"pp{bi}",
                                 name=f"ps_P{bi}") for bi in range(B)]

            def pmm(k, bi):
                nc.tensor.matmul(
                    ps_Ps[bi][:],
                    xt[:, k * B + bi, :, :],
                    at[:, k, :, :],
                    start=(k == 0), stop=(k == KP - 1),
                    perf_mode=DR, skip_group_check=True,
                )

            for k in range(KP - 1):
                for bi in range(B):
                    pmm(k, bi)
            # last pair batch-by-batch, chaining each batch's epilogue in
            # right behind its final accumulation
            for bi in range(B):
                pmm(KP - 1, bi)
                psb = psbp.tile([128, SLICE], BF16, tag="psb", name=f"psb{bi}")
                nc.vector.tensor_mul(psb[:], ps_Ps[bi][:], recipf[:])
                ps_y = ps_y_p.tile([128, SLICE], F32, tag="py",
                                   name=f"ps_y{bi}")
                nc.tensor.matmul(ps_y[:], w1t[:], psb[:],
                                 start=True, stop=False, skip_group_check=True)
                nc.tensor.matmul(ps_y[:], w0t[:],
                                 xs[:, bi * SLICE:(bi + 1) * SLICE],
                                 start=False, stop=False, skip_group_check=True)
                nc.tensor.matmul(ps_y[:], blhs[:], brhs[:],
                                 start=False, stop=True, skip_group_check=True)
                ost = ostp.tile([O, SLICE], BF16, tag="ost", name=f"ost{bi}")
                nc.vector.tensor_copy(ost[:], ps_y[:])
                nc.sync.dma_start(youts[bi][:], ost[:])

    nc.compile()
    return nc


def kernel(x, W, b, idx_i, idx_j):
    in_maps = prep_inputs(x, W, b, idx_i, idx_j)
    nc = build_program()
    res = run_bass_kernel_spmd(nc, in_maps, list(range(NCORES)))
    y = np.empty((B, O, N), np.float32)
    for s in range(NCORES):
        for bi in range(B):
            y[bi, :, s * SLICE:(s + 1) * SLICE] = \
                res.results[s][f"y{bi}"].astype(np.float32)
    return y


if __name__ == "__main__":
    rng = np.random.default_rng(0)
    x = rng.standard_normal((B, C, N), np.float32)
    W = rng.standard_normal((127, C, 2), np.float32) * 0.05
    b = rng.standard_normal((127,), np.float32) * 0.05
    idx_i = rng.integers(0, N, 131072)
    idx_j = rng.integers(0, N, 131072)
    y = kernel(x, W, b, idx_i, idx_j)
    print("ok", y.shape, float(np.abs(y).mean()))


# revision 15
# speedup vs baseline: 1.0006x; 1.0006x over previous
"""Trainium2 Bass kernel for nn_PairwiseConv (gnn_message_passing).

Reference computation, for each edge e=(i,j) of a sparse adjacency:
    pair[b,o,e] = sum_c W[o,c,0]*x[b,c,i] + W[o,c,1]*x[b,c,j] + bias[o]
    y[b,o,n]    = (sum_{e: i_e=n} pair[b,o,e]) / max(deg_j[n],1)
    y[b,127,n]  = deg_j[n]            (counts channel)
where deg_j[n] = #{e: j_e = n}.

Algebraic reformulation (exact):
    y[b,o,n] = S[b,o,n]*recip[n] + (W0^T x)[b,o,n]*c1[n] + bias[o]*c1[n]
    S        = W1^T (x @ AT),  AT[m,n] = #{e: j_e=m, i_e=n}
    recip[n] = 1/max(deg_j[n],1),  c1[n] = deg_i[n]*recip[n]
Key trick: contract x against the count matrix FIRST (P = x @ AT), then
apply the 128x128 conv weights to the much smaller P. The only heavy
matmul is P, done in fp8(e4m3) DoubleRow mode (256-row contraction per
pass) -- counts are small ints (exact in fp8) and the fp8 error on x
only touches the minority S term of the output.

Sharding: 8 cores x 512 dst-node slices, all 4 batches per core.

Device schedule (per core):
 - sync HWDGE queue: AT pair-blocks, small-first so phase P starts early;
   output DMAs at the end.
 - scalar HWDGE queue: tiny consts, x^T front half pair-blocks, then the
   c1-prescaled bf16 x-slice for the W0 pass.
 - gpsimd (software) queue: x^T back half as one big-row DMA (the
   software DGE is descriptor-rate-bound, so one 8KB-row transfer only).
 - PE: dummy fp8 warm-up matmuls during the DMA wait (p-state ramp),
   1-row f32r broadcast of recip, then 64 fp8 DoubleRow phase-P matmuls;
   per batch a bf16 W1 pass on P*recip, a bf16 W0 pass on the x-slice,
   and a 2-row f32r bias/deg pass, all accumulating in one PSUM bank
   that is then copied out as bf16.
"""

import numpy as np
import ml_dtypes

import concourse.bass as bass
import concourse.mybir as mybir
import concourse.tile as tile
from concourse import bacc
from concourse.bass_utils import run_bass_kernel_spmd

B = 4
C = 128   # in channels
O = 128   # out channels incl. counts row (127 real + zero row)
N = 4096
SLICE = 512   # dst nodes per core
NCORES = 8
MC = 32       # 128-row source chunks
KP = MC // 2  # chunk pairs (DoubleRow)
F32 = mybir.dt.float32
F32R = mybir.dt.float32r  # unused: f32r paths failed neuronx compile
BF16 = mybir.dt.bfloat16
F8 = mybir.dt.float8e4
BF16_NP = ml_dtypes.bfloat16
F8_NP = ml_dtypes.float8_e4m3
DR = mybir.MatmulPerfMode.DoubleRow

# pair-blocks per DMA: small first (early PE start), bigger later
AT_SYNC = [(0, 2), (2, 4), (4, 8), (8, 12)]    # on sync
AT_SCALAR = (12, 16)                           # tail block on scalar
XT_BLOCKS = [(0, 2), (2, 4), (4, 6), (6, 8)]   # front half on scalar
XT_GP = (8, 16)                                # back half on gpsimd


def prep_inputs(x, W, b, idx_i, idx_j):
    x = np.ascontiguousarray(np.asarray(x, np.float32))
    W = np.asarray(W, np.float32)
    bias = np.asarray(b, np.float32)
    ii = np.asarray(idx_i).astype(np.int64)
    jj = np.asarray(idx_j).astype(np.int64)

    # x^T pair-major [p=m%128, (kp, b, t, c)] -- same for all cores
    # xt8[p, k, b, t, c] = x[b, c, (2k+t)*128 + p]
    xt8 = np.ascontiguousarray(
        x.transpose(2, 0, 1)              # [N, B, C]
        .reshape(KP, 2, 128, B, C)        # [k, t, p, b, c]
        .transpose(2, 0, 3, 1, 4)         # [p, k, b, t, c]
        .reshape(128, KP * B, 2, C)
    ).astype(F8_NP)

    # conv weights as lhsT [c, o], o=127 padded with a zero column
    W0T = np.zeros((128, 128), BF16_NP)
    W0T[:, :127] = W[:, :, 0].T.astype(BF16_NP)
    W1T = np.zeros((128, 128), BF16_NP)
    W1T[:, :127] = W[:, :, 1].T.astype(BF16_NP)

    deg_j = np.bincount(jj, minlength=N).astype(np.float32)
    deg_i = np.bincount(ii, minlength=N).astype(np.float32)
    maxdj = np.maximum(deg_j, 1.0)
    recip = (1.0 / maxdj).astype(np.float32)
    c1 = (deg_i / maxdj).astype(np.float32)

    # bias/deg rank-2 pass: lhsT [2, 128] (bias row + counts-channel row)
    blhs = np.zeros((2, 128), np.float32)
    blhs[0, :127] = bias
    blhs[1, 127] = 1.0
    blhs = blhs.astype(BF16_NP)

    in_maps = []
    for s in range(NCORES):
        base = s * SLICE
        sel = (ii >= base) & (ii < base + SLICE)
        atf = np.zeros((N, SLICE), np.float32)
        np.add.at(atf, (jj[sel], ii[sel] - base), 1.0)
        at8 = np.ascontiguousarray(
            atf.reshape(KP, 2, 128, SLICE).transpose(2, 0, 1, 3)
            .reshape(128, KP, 2, SLICE)
        ).astype(F8_NP)

        rs = recip[base:base + SLICE]
        c1s = c1[base:base + SLICE]
        # bf16 x slice pre-scaled by c1 (covers the deg_i*(W0^T x) term)
        xs = np.ascontiguousarray(
            (x[:, :, base:base + SLICE] * c1s[None, None, :])
            .transpose(1, 0, 2).reshape(128, B * SLICE)
        ).astype(BF16_NP)
        brhs = np.stack([c1s, deg_j[base:base + SLICE]]).astype(BF16_NP)

        in_maps.append({
            "XT8": xt8, "AT8": at8, "XS": xs,
            "RROW": rs.reshape(1, SLICE).astype(BF16_NP), "BLHS": blhs, "BRHS": brhs,
            "W0T": W0T, "W1T": W1T,
        })
    return in_maps


def build_program():
    nc = bacc.Bacc("TRN2", target_bir_lowering=False, debug=False,
                   num_devices=NCORES)

    XT8 = nc.dram_tensor("XT8", [128, KP * B, 2, C], F8, kind="ExternalInput")
    AT8 = nc.dram_tensor("AT8", [128, KP, 2, SLICE], F8, kind="ExternalInput")
    XS = nc.dram_tensor("XS", [128, B * SLICE], BF16, kind="ExternalInput")
    RROW = nc.dram_tensor("RROW", [1, SLICE], BF16, kind="ExternalInput")
    BLHS = nc.dram_tensor("BLHS", [2, 128], BF16, kind="ExternalInput")
    BRHS = nc.dram_tensor("BRHS", [2, SLICE], BF16, kind="ExternalInput")
    W0T = nc.dram_tensor("W0T", [128, 128], BF16, kind="ExternalInput")
    W1T = nc.dram_tensor("W1T", [128, 128], BF16, kind="ExternalInput")
    youts = [nc.dram_tensor(f"y{bi}", [O, SLICE], BF16, kind="ExternalOutput")
             for bi in range(B)]

    with tile.TileContext(nc) as tc:
        with (
            tc.tile_pool(name="big", bufs=1) as bigp,
            tc.tile_pool(name="const", bufs=1) as constp,
            tc.tile_pool(name="psb", bufs=2) as psbp,
            tc.tile_pool(name="ost", bufs=2) as ostp,
            tc.tile_pool(name="ps_P", bufs=1, space="PSUM") as ps_P_p,
            tc.tile_pool(name="ps_y", bufs=2, space="PSUM") as ps_y_p,
            tc.tile_pool(name="ps_w", bufs=1, space="PSUM") as ps_w_p,
        ):
            at = bigp.tile([128, KP, 2, SLICE], F8)
            xt = bigp.tile([128, KP * B, 2, C], F8)
            # tiny consts first on scalar (1-2 descriptors each)
            rrow = constp.tile([1, SLICE], BF16)
            nc.scalar.dma_start(rrow[:], RROW[:])
            blhs = constp.tile([2, 128], BF16)
            nc.scalar.dma_start(blhs[:], BLHS[:])
            brhs = constp.tile([2, SLICE], BF16)
            nc.scalar.dma_start(brhs[:], BRHS[:])
            # main streams
            for lo, hi in AT_SYNC:
                nc.sync.dma_start(at[:, lo:hi, :, :], AT8[:, lo:hi, :, :])
            for lo, hi in XT_BLOCKS:
                nc.scalar.dma_start(xt[:, lo * B:hi * B, :, :],
                                    XT8[:, lo * B:hi * B, :, :])
            lo, hi = XT_GP
            nc.gpsimd.dma_start(xt[:, lo * B:hi * B, :, :],
                                XT8[:, lo * B:hi * B, :, :])
            # late-needed data after the phase-P streams
            lo, hi = AT_SCALAR
            nc.scalar.dma_start(at[:, lo:hi, :, :], AT8[:, lo:hi, :, :])
            xs = constp.tile([128, B * SLICE], BF16)
            nc.scalar.dma_start(xs[:], XS[:])
            w0t = constp.tile([128, 128], BF16)
            nc.scalar.dma_start(w0t[:], W0T[:])
            w1t = constp.tile([128, 128], BF16)
            nc.scalar.dma_start(w1t[:], W1T[:])

            # PE warm-up on memset tiles while DMA streams in (p-state ramp)
            wlhs = constp.tile([128, 2, C], F8)
            nc.vector.memset(wlhs[:], 0.0)
            wrhs = constp.tile([128, 2, SLICE], F8)
            nc.vector.memset(wrhs[:], 0.0)
            ps_w = ps_w_p.tile([128, SLICE], F32, tag="warm", name="ps_warm")
            for _ in range(10):
                nc.tensor.matmul(ps_w[:], wlhs[:], wrhs[:],
                                 start=True, stop=True,
                                 perf_mode=DR, skip_group_check=True)
            # broadcast recip row to all 128 partitions via 1-row matmul
            ones1 = constp.tile([1, 128], BF16)
            nc.vector.memset(ones1[:], 1.0)
            ps_r = ps_w_p.tile([128, SLICE], F32, tag="rbc", name="ps_rbc")
            nc.tensor.matmul(ps_r[:], ones1[:], rrow[:], start=True, stop=True,
                             skip_group_check=True)
            recipf = constp.tile([128, SLICE], BF16)
            nc.vector.tensor_copy(recipf[:], ps_r[:])

            # phase P: ps_P[b] += xT[pair k, b]^T @ AT[pair k] (fp8 DoubleRow)
            ps_Ps = [ps_P_p.tile([128, SLICE], F32, tag=f"pp{bi}",
                                 name=f"ps_P{bi}") for bi in range(B)]

            def pmm(k, bi):
                nc.tensor.matmul(
                    ps_Ps[bi][:],
                    xt[:, k * B + bi, :, :],
                    at[:, k, :, :],
                    start=(k == 0), stop=(k == KP - 1),
                    perf_mode=DR, skip_group_check=True,
                )

            for k in range(KP):
                for bi in range(B):
                    pmm(k, bi)
            # epilogue, batch-pipelined: vector scales P while the PE runs
            # the next batch's passes; scalar copies results out
            for bi in range(B):
                psb = psbp.tile([128, SLICE], BF16, tag="psb", name=f"psb{bi}")
                nc.vector.tensor_mul(psb[:], ps_Ps[bi][:], recipf[:])
                ps_y = ps_y_p.tile([128, SLICE], F32, tag="py",
                                   name=f"ps_y{bi}")
                nc.tensor.matmul(ps_y[:], w1t[:], psb[:],
                                 start=True, stop=False, skip_group_check=True)
                nc.tensor.matmul(ps_y[:], w0t[:],
                                 xs[:, bi * SLICE:(bi + 1) * SLICE],
                                 start=False, stop=False, skip_group_check=True)
                nc.tensor.matmul(ps_y[:], blhs[:], brhs[:],
                                 start=False, stop=True, skip_group_check=True)
                ost = ostp.tile([O, SLICE], BF16, tag="ost", name=f"ost{bi}")
                nc.scalar.copy(ost[:], ps_y[:])
                nc.sync.dma_start(youts[bi][:], ost[:])

    nc.compile()
    return nc


def kernel(x, W, b, idx_i, idx_j):
    in_maps = prep_inputs(x, W, b, idx_i, idx_j)
    nc = build_program()
    res = run_bass_kernel_spmd(nc, in_maps, list(range(NCORES)))
    y = np.empty((B, O, N), np.float32)
    for s in range(NCORES):
        for bi in range(B):
            y[bi, :, s * SLICE:(s + 1) * SLICE] = \
                res.results[s][f"y{bi}"].astype(np.float32)
    return y


if __name__ == "__main__":
    rng = np.random.default_rng(0)
    x = rng.standard_normal((B, C, N), np.float32)
    W = rng.standard_normal((127, C, 2), np.float32) * 0.05
    b = rng.standard_normal((127,), np.float32) * 0.05
    idx_i = rng.integers(0, N, 131072)
    idx_j = rng.integers(0, N, 131072)
    y = kernel(x, W, b, idx_i, idx_j)
    print("ok", y.shape, float(np.abs(y).mean()))


# revision 17
# speedup vs baseline: 1.0374x; 1.0368x over previous
"""Trainium2 Bass kernel for nn_PairwiseConv (gnn_message_passing).

Reference computation, for each edge e=(i,j) of a sparse adjacency:
    pair[b,o,e] = sum_c W[o,c,0]*x[b,c,i] + W[o,c,1]*x[b,c,j] + bias[o]
    y[b,o,n]    = (sum_{e: i_e=n} pair[b,o,e]) / max(deg_j[n],1)
    y[b,127,n]  = deg_j[n]            (counts channel)
where deg_j[n] = #{e: j_e = n}.

Algebraic reformulation (exact):
    y[b,o,n] = S[b,o,n]*recip[n] + (W0^T x)[b,o,n]*c1[n] + bias[o]*c1[n]
    S        = W1^T (x @ AT),  AT[m,n] = #{e: j_e=m, i_e=n}
    recip[n] = 1/max(deg_j[n],1),  c1[n] = deg_i[n]*recip[n]
Key trick: contract x against the count matrix FIRST (P = x @ AT), then
apply the 128x128 conv weights to the much smaller P. The only heavy
matmul is P, done in fp8(e4m3) DoubleRow mode (256-row contraction per
pass) -- counts are small ints (exact in fp8) and the fp8 error on x
only touches the minority S term of the output.

Sharding: 8 cores x 512 dst-node slices, all 4 batches per core.

Device schedule (per core):
 - sync HWDGE queue: AT pair-blocks, small-first so phase P starts early;
   output DMAs at the end.
 - scalar HWDGE queue: tiny consts, x^T front half pair-blocks, then the
   c1-prescaled bf16 x-slice for the W0 pass.
 - gpsimd (software) queue: x^T back half as one big-row DMA (the
   software DGE is descriptor-rate-bound, so one 8KB-row transfer only).
 - PE: dummy fp8 warm-up matmuls during the DMA wait (p-state ramp),
   1-row f32r broadcast of recip, then 64 fp8 DoubleRow phase-P matmuls;
   per batch a bf16 W1 pass on P*recip, a bf16 W0 pass on the x-slice,
   and a 2-row f32r bias/deg pass, all accumulating in one PSUM bank
   that is then copied out as bf16.
"""

import numpy as np
import ml_dtypes

import concourse.bass as bass
import concourse.mybir as mybir
import concourse.tile as tile
from concourse import bacc
from concourse.bass_utils import run_bass_kernel_spmd

B = 4
C = 128   # in channels
O = 128   # out channels incl. counts row (127 real + zero row)
N = 4096
SLICE = 512   # dst nodes per core
NCORES = 8
MC = 32       # 128-row source chunks
KP = MC // 2  # chunk pairs (DoubleRow)
F32 = mybir.dt.float32
F32R = mybir.dt.float32r  # unused: f32r paths failed neuronx compile
BF16 = mybir.dt.bfloat16
F8 = mybir.dt.float8e4
BF16_NP = ml_dtypes.bfloat16
F8_NP = ml_dtypes.float8_e4m3
DR = mybir.MatmulPerfMode.DoubleRow

# pair-blocks per DMA: small first (early PE start), bigger later.
# Nothing rides the gpsimd software queue -- its descriptor generation
# contends with the hardware DGE queues and stalls them.
AT_SYNC = [(0, 2), (2, 4), (4, 8), (8, 12), (12, 16)]
XT_BLOCKS = [(0, 2), (2, 4), (4, 8), (8, 12), (12, 16)]


def prep_inputs(x, W, b, idx_i, idx_j):
    x = np.ascontiguousarray(np.asarray(x, np.float32))
    W = np.asarray(W, np.float32)
    bias = np.asarray(b, np.float32)
    ii = np.asarray(idx_i).astype(np.int64)
    jj = np.asarray(idx_j).astype(np.int64)

    # x^T pair-major [p=m%128, (kp, b, t, c)] -- same for all cores
    # xt8[p, k, b, t, c] = x[b, c, (2k+t)*128 + p]
    xt8 = np.ascontiguousarray(
        x.transpose(2, 0, 1)              # [N, B, C]
        .reshape(KP, 2, 128, B, C)        # [k, t, p, b, c]
        .transpose(2, 0, 3, 1, 4)         # [p, k, b, t, c]
        .reshape(128, KP * B, 2, C)
    ).astype(F8_NP)

    # conv weights as lhsT [c, o], o=127 padded with a zero column
    W0T = np.zeros((128, 128), BF16_NP)
    W0T[:, :127] = W[:, :, 0].T.astype(BF16_NP)
    W1T = np.zeros((128, 128), BF16_NP)
    W1T[:, :127] = W[:, :, 1].T.astype(BF16_NP)

    deg_j = np.bincount(jj, minlength=N).astype(np.float32)
    deg_i = np.bincount(ii, minlength=N).astype(np.float32)
    maxdj = np.maximum(deg_j, 1.0)
    recip = (1.0 / maxdj).astype(np.float32)
    c1 = (deg_i / maxdj).astype(np.float32)

    # bias/deg rank-2 pass: lhsT [2, 128] (bias row + counts-channel row)
    blhs = np.zeros((2, 128), np.float32)
    blhs[0, :127] = bias
    blhs[1, 127] = 1.0
    blhs = blhs.astype(BF16_NP)

    in_maps = []
    for s in range(NCORES):
        base = s * SLICE
        sel = (ii >= base) & (ii < base + SLICE)
        atf = np.zeros((N, SLICE), np.float32)
        np.add.at(atf, (jj[sel], ii[sel] - base), 1.0)
        at8 = np.ascontiguousarray(
            atf.reshape(KP, 2, 128, SLICE).transpose(2, 0, 1, 3)
            .reshape(128, KP, 2, SLICE)
        ).astype(F8_NP)

        rs = recip[base:base + SLICE]
        c1s = c1[base:base + SLICE]
        # bf16 x slice pre-scaled by c1 (covers the deg_i*(W0^T x) term)
        xs = np.ascontiguousarray(
            (x[:, :, base:base + SLICE] * c1s[None, None, :])
            .transpose(1, 0, 2).reshape(128, B * SLICE)
        ).astype(BF16_NP)
        brhs = np.stack([c1s, deg_j[base:base + SLICE]]).astype(BF16_NP)

        in_maps.append({
            "XT8": xt8, "AT8": at8, "XS": xs,
            "RROW": rs.reshape(1, SLICE).astype(BF16_NP), "BLHS": blhs, "BRHS": brhs,
            "W0T": W0T, "W1T": W1T,
        })
    return in_maps


def build_program():
    nc = bacc.Bacc("TRN2", target_bir_lowering=False, debug=False,
                   num_devices=NCORES)

    XT8 = nc.dram_tensor("XT8", [128, KP * B, 2, C], F8, kind="ExternalInput")
    AT8 = nc.dram_tensor("AT8", [128, KP, 2, SLICE], F8, kind="ExternalInput")
    XS = nc.dram_tensor("XS", [128, B * SLICE], BF16, kind="ExternalInput")
    RROW = nc.dram_tensor("RROW", [1, SLICE], BF16, kind="ExternalInput")
    BLHS = nc.dram_tensor("BLHS", [2, 128], BF16, kind="ExternalInput")
    BRHS = nc.dram_tensor("BRHS", [2, SLICE], BF16, kind="ExternalInput")
    W0T = nc.dram_tensor("W0T", [128, 128], BF16, kind="ExternalInput")
    W1T = nc.dram_tensor("W1T", [128, 128], BF16, kind="ExternalInput")
    youts = [nc.dram_tensor(f"y{bi}", [O, SLICE], BF16, kind="ExternalOutput")
             for bi in range(B)]

    with tile.TileContext(nc) as tc:
        with (
            tc.tile_pool(name="big", bufs=1) as bigp,
            tc.tile_pool(name="const", bufs=1) as constp,
            tc.tile_pool(name="psb", bufs=2) as psbp,
            tc.tile_pool(name="ost", bufs=2) as ostp,
            tc.tile_pool(name="ps_P", bufs=1, space="PSUM") as ps_P_p,
            tc.tile_pool(name="ps_y", bufs=2, space="PSUM") as ps_y_p,
            tc.tile_pool(name="ps_w", bufs=1, space="PSUM") as ps_w_p,
        ):
            at = bigp.tile([128, KP, 2, SLICE], F8)
            xt = bigp.tile([128, KP * B, 2, C], F8)
            # tiny consts first on scalar (1-2 descriptors each)
            rrow = constp.tile([1, SLICE], BF16)
            nc.scalar.dma_start(rrow[:], RROW[:])
            blhs = constp.tile([2, 128], BF16)
            nc.scalar.dma_start(blhs[:], BLHS[:])
            brhs = constp.tile([2, SLICE], BF16)
            nc.scalar.dma_start(brhs[:], BRHS[:])
            # main streams: AT on sync, x^T on scalar, pair-block-major
            for lo, hi in AT_SYNC:
                nc.sync.dma_start(at[:, lo:hi, :, :], AT8[:, lo:hi, :, :])
            for lo, hi in XT_BLOCKS:
                nc.scalar.dma_start(xt[:, lo * B:hi * B, :, :],
                                    XT8[:, lo * B:hi * B, :, :])
            # late-needed data behind the phase-P streams
            xs = constp.tile([128, B * SLICE], BF16)
            nc.sync.dma_start(xs[:], XS[:])
            w0t = constp.tile([128, 128], BF16)
            nc.scalar.dma_start(w0t[:], W0T[:])
            w1t = constp.tile([128, 128], BF16)
            nc.scalar.dma_start(w1t[:], W1T[:])

            # PE warm-up on memset tiles while DMA streams in (p-state ramp)
            wlhs = constp.tile([128, 2, C], F8)
            nc.vector.memset(wlhs[:], 0.0)
            wrhs = constp.tile([128, 2, SLICE], F8)
            nc.vector.memset(wrhs[:], 0.0)
            ps_w = ps_w_p.tile([128, SLICE], F32, tag="warm", name="ps_warm")
            for _ in range(10):
                nc.tensor.matmul(ps_w[:], wlhs[:], wrhs[:],
                                 start=True, stop=True,
                                 perf_mode=DR, skip_group_check=True)
            # broadcast recip row to all 128 partitions via 1-row matmul
            ones1 = constp.tile([1, 128], BF16)
            nc.vector.memset(ones1[:], 1.0)
            ps_r = ps_w_p.tile([128, SLICE], F32, tag="rbc", name="ps_rbc")
            nc.tensor.matmul(ps_r[:], ones1[:], rrow[:], start=True, stop=True,
                             skip_group_check=True)
            recipf = constp.tile([128, SLICE], BF16)
            nc.vector.tensor_copy(recipf[:], ps_r[:])

            # phase P: ps_P[b] += xT[pair k, b]^T @ AT[pair k] (fp8 DoubleRow)
            ps_Ps = [ps_P_p.tile([128, SLICE], F32, tag=f"pp{bi}",
                                 name=f"ps_P{bi}") for bi in range(B)]

            def pmm(k, bi):
                nc.tensor.matmul(
                    ps_Ps[bi][:],
                    xt[:, k * B + bi, :, :],
                    at[:, k, :, :],
                    start=(k == 0), stop=(k == KP - 1),
                    perf_mode=DR, skip_group_check=True,
                )

            for k in range(KP):
                for bi in range(B):
                    pmm(k, bi)
            # epilogue, batch-pipelined: vector scales P while the PE runs
            # the next batch's passes; scalar copies results out
            for bi in range(B):
                psb = psbp.tile([128, SLICE], BF16, tag="psb", name=f"psb{bi}")
                nc.vector.tensor_mul(psb[:], ps_Ps[bi][:], recipf[:])
                ps_y = ps_y_p.tile([128, SLICE], F32, tag="py",
                                   name=f"ps_y{bi}")
                nc.tensor.matmul(ps_y[:], w1t[:], psb[:],
                                 start=True, stop=False, skip_group_check=True)
                nc.tensor.matmul(ps_y[:], w0t[:],
                                 xs[:, bi * SLICE:(bi + 1) * SLICE],
                                 start=False, stop=False, skip_group_check=True)
                nc.tensor.matmul(ps_y[:], blhs[:], brhs[:],
                                 start=False, stop=True, skip_group_check=True)
                ost = ostp.tile([O, SLICE], BF16, tag="ost", name=f"ost{bi}")
                nc.scalar.copy(ost[:], ps_y[:])
                nc.sync.dma_start(youts[bi][:], ost[:])

    nc.compile()
    return nc


def kernel(x, W, b, idx_i, idx_j):
    in_maps = prep_inputs(x, W, b, idx_i, idx_j)
    nc = build_program()
    res = run_bass_kernel_spmd(nc, in_maps, list(range(NCORES)))
    y = np.empty((B, O, N), np.float32)
    for s in range(NCORES):
        for bi in range(B):
            y[bi, :, s * SLICE:(s + 1) * SLICE] = \
                res.results[s][f"y{bi}"].astype(np.float32)
    return y


if __name__ == "__main__":
    rng = np.random.default_rng(0)
    x = rng.standard_normal((B, C, N), np.float32)
    W = rng.standard_normal((127, C, 2), np.float32) * 0.05
    b = rng.standard_normal((127,), np.float32) * 0.05
    idx_i = rng.integers(0, N, 131072)
    idx_j = rng.integers(0, N, 131072)
    y = kernel(x, W, b, idx_i, idx_j)
    print("ok", y.shape, float(np.abs(y).mean()))


# revision 18
# speedup vs baseline: 1.1178x; 1.0775x over previous
"""Trainium2 Bass kernel for nn_PairwiseConv (gnn_message_passing).

Reference computation, for each edge e=(i,j) of a sparse adjacency:
    pair[b,o,e] = sum_c W[o,c,0]*x[b,c,i] + W[o,c,1]*x[b,c,j] + bias[o]
    y[b,o,n]    = (sum_{e: i_e=n} pair[b,o,e]) / max(deg_j[n],1)
    y[b,127,n]  = deg_j[n]            (counts channel)
where deg_j[n] = #{e: j_e = n}.

Algebraic reformulation (exact):
    y[b,o,n] = S[b,o,n]*recip[n] + (W0^T x)[b,o,n]*c1[n] + bias[o]*c1[n]
    S        = W1^T (x @ AT),  AT[m,n] = #{e: j_e=m, i_e=n}
    recip[n] = 1/max(deg_j[n],1),  c1[n] = deg_i[n]*recip[n]
Key trick: contract x against the count matrix FIRST (P = x @ AT), then
apply the 128x128 conv weights to the much smaller P. The only heavy
matmul is P, done in fp8(e4m3) DoubleRow mode (256-row contraction per
pass) -- counts are small ints (exact in fp8) and the fp8 error on x
only touches the minority S term of the output.

Sharding: 8 cores x 512 dst-node slices, all 4 batches per core.

Device schedule (per core):
 - sync HWDGE queue: AT pair-blocks, small-first so phase P starts early;
   output DMAs at the end.
 - scalar HWDGE queue: tiny consts, x^T front half pair-blocks, then the
   c1-prescaled bf16 x-slice for the W0 pass.
 - gpsimd (software) queue: x^T back half as one big-row DMA (the
   software DGE is descriptor-rate-bound, so one 8KB-row transfer only).
 - PE: dummy fp8 warm-up matmuls during the DMA wait (p-state ramp),
   1-row f32r broadcast of recip, then 64 fp8 DoubleRow phase-P matmuls;
   per batch a bf16 W1 pass on P*recip, a bf16 W0 pass on the x-slice,
   and a 2-row f32r bias/deg pass, all accumulating in one PSUM bank
   that is then copied out as bf16.
"""

import numpy as np
import ml_dtypes

import concourse.bass as bass
import concourse.mybir as mybir
import concourse.tile as tile
from concourse import bacc
from concourse.bass_utils import run_bass_kernel_spmd

B = 4
C = 128   # in channels
O = 128   # out channels incl. counts row (127 real + zero row)
N = 4096
SLICE = 512   # dst nodes per core
NCORES = 8
MC = 32       # 128-row source chunks
KP = MC // 2  # chunk pairs (DoubleRow)
F32 = mybir.dt.float32
F32R = mybir.dt.float32r  # unused: f32r paths failed neuronx compile
BF16 = mybir.dt.bfloat16
F8 = mybir.dt.float8e4
BF16_NP = ml_dtypes.bfloat16
F8_NP = ml_dtypes.float8_e4m3
DR = mybir.MatmulPerfMode.DoubleRow

# pair-blocks per DMA: small first (early PE start), bigger later.
# Nothing rides the gpsimd software queue -- its descriptor generation
# contends with the hardware DGE queues and stalls them.
AT_SYNC = [(0, 2), (2, 4), (4, 8), (8, 12), (12, 16)]
XT_BLOCKS = [(0, 2), (2, 4), (4, 8), (8, 12), (12, 16)]


def prep_inputs(x, W, b, idx_i, idx_j):
    x = np.ascontiguousarray(np.asarray(x, np.float32))
    W = np.asarray(W, np.float32)
    bias = np.asarray(b, np.float32)
    ii = np.asarray(idx_i).astype(np.int64)
    jj = np.asarray(idx_j).astype(np.int64)

    # x^T pair-major [p=m%128, (kp, b, t, c)] -- same for all cores
    # xt8[p, k, b, t, c] = x[b, c, (2k+t)*128 + p]
    xt8 = np.ascontiguousarray(
        x.transpose(2, 0, 1)              # [N, B, C]
        .reshape(KP, 2, 128, B, C)        # [k, t, p, b, c]
        .transpose(2, 0, 3, 1, 4)         # [p, k, b, t, c]
        .reshape(128, KP * B, 2, C)
    ).astype(F8_NP)

    # conv weights as lhsT [c, o], o=127 padded with a zero column;
    # both kernels packed in one tensor (one DMA, 512B rows)
    W01 = np.zeros((128, 256), BF16_NP)
    W01[:, :127] = W[:, :, 0].T.astype(BF16_NP)
    W01[:, 128:255] = W[:, :, 1].T.astype(BF16_NP)

    deg_j = np.bincount(jj, minlength=N).astype(np.float32)
    deg_i = np.bincount(ii, minlength=N).astype(np.float32)
    maxdj = np.maximum(deg_j, 1.0)
    recip = (1.0 / maxdj).astype(np.float32)
    c1 = (deg_i / maxdj).astype(np.float32)

    # bias/deg rank-2 pass: lhsT [2, 128] (bias row + counts-channel row)
    blhs = np.zeros((2, 128), np.float32)
    blhs[0, :127] = bias
    blhs[1, 127] = 1.0
    blhs = blhs.astype(BF16_NP)

    in_maps = []
    for s in range(NCORES):
        base = s * SLICE
        sel = (ii >= base) & (ii < base + SLICE)
        atf = np.zeros((N, SLICE), np.float32)
        np.add.at(atf, (jj[sel], ii[sel] - base), 1.0)
        at8 = np.ascontiguousarray(
            atf.reshape(KP, 2, 128, SLICE).transpose(2, 0, 1, 3)
            .reshape(128, KP, 2, SLICE)
        ).astype(F8_NP)

        rs = recip[base:base + SLICE]
        c1s = c1[base:base + SLICE]
        # bf16 x slice pre-scaled by c1 (covers the deg_i*(W0^T x) term)
        xs = np.ascontiguousarray(
            (x[:, :, base:base + SLICE] * c1s[None, None, :])
            .transpose(1, 0, 2).reshape(128, B * SLICE)
        ).astype(BF16_NP)
        brhs = np.stack([c1s, deg_j[base:base + SLICE]]).astype(BF16_NP)

        in_maps.append({
            "XT8": xt8, "AT8": at8, "XS": xs,
            "RROW": rs.reshape(1, SLICE).astype(BF16_NP), "BLHS": blhs, "BRHS": brhs,
            "W01": W01,
        })
    return in_maps


def build_program():
    nc = bacc.Bacc("TRN2", target_bir_lowering=False, debug=False,
                   num_devices=NCORES)

    XT8 = nc.dram_tensor("XT8", [128, KP * B, 2, C], F8, kind="ExternalInput")
    AT8 = nc.dram_tensor("AT8", [128, KP, 2, SLICE], F8, kind="ExternalInput")
    XS = nc.dram_tensor("XS", [128, B * SLICE], BF16, kind="ExternalInput")
    RROW = nc.dram_tensor("RROW", [1, SLICE], BF16, kind="ExternalInput")
    BLHS = nc.dram_tensor("BLHS", [2, 128], BF16, kind="ExternalInput")
    BRHS = nc.dram_tensor("BRHS", [2, SLICE], BF16, kind="ExternalInput")
    W01 = nc.dram_tensor("W01", [128, 256], BF16, kind="ExternalInput")
    youts = [nc.dram_tensor(f"y{bi}", [O, SLICE], BF16, kind="ExternalOutput")
             for bi in range(B)]

    with tile.TileContext(nc) as tc:
        with (
            tc.tile_pool(name="big", bufs=1) as bigp,
            tc.tile_pool(name="const", bufs=1) as constp,
            tc.tile_pool(name="psb", bufs=2) as psbp,
            tc.tile_pool(name="ost", bufs=2) as ostp,
            tc.tile_pool(name="ps_P", bufs=1, space="PSUM") as ps_P_p,
            tc.tile_pool(name="ps_y", bufs=2, space="PSUM") as ps_y_p,
            tc.tile_pool(name="ps_w", bufs=1, space="PSUM") as ps_w_p,
        ):
            at = bigp.tile([128, KP, 2, SLICE], F8)
            xt = bigp.tile([128, KP * B, 2, C], F8)
            # tiny consts (1-2 descriptors each) on gpsimd: negligible
            # software-DGE work, keeps the HWDGE queues clean
            rrow = constp.tile([1, SLICE], BF16)
            nc.gpsimd.dma_start(rrow[:], RROW[:])
            blhs = constp.tile([2, 128], BF16)
            nc.gpsimd.dma_start(blhs[:], BLHS[:])
            brhs = constp.tile([2, SLICE], BF16)
            nc.gpsimd.dma_start(brhs[:], BRHS[:])
            # main streams: AT (+x-slice) on sync, weights + x^T on scalar
            w01 = constp.tile([128, 256], BF16)
            nc.scalar.dma_start(w01[:], W01[:])
            for lo, hi in AT_SYNC:
                nc.sync.dma_start(at[:, lo:hi, :, :], AT8[:, lo:hi, :, :])
            for lo, hi in XT_BLOCKS:
                nc.scalar.dma_start(xt[:, lo * B:hi * B, :, :],
                                    XT8[:, lo * B:hi * B, :, :])
            # late-needed data behind the phase-P streams
            xs = constp.tile([128, B * SLICE], BF16)
            nc.sync.dma_start(xs[:], XS[:])
            w0t = w01[:, 0:128]
            w1t = w01[:, 128:256]

            # PE warm-up on memset tiles while DMA streams in (p-state ramp)
            wlhs = constp.tile([128, 2, C], F8)
            nc.vector.memset(wlhs[:], 0.0)
            wrhs = constp.tile([128, 2, SLICE], F8)
            nc.vector.memset(wrhs[:], 0.0)
            ps_w = ps_w_p.tile([128, SLICE], F32, tag="warm", name="ps_warm")
            for _ in range(10):
                nc.tensor.matmul(ps_w[:], wlhs[:], wrhs[:],
                                 start=True, stop=True,
                                 perf_mode=DR, skip_group_check=True)
            # broadcast recip row to all 128 partitions via 1-row matmul
            ones1 = constp.tile([1, 128], BF16)
            nc.vector.memset(ones1[:], 1.0)
            ps_r = ps_w_p.tile([128, SLICE], F32, tag="rbc", name="ps_rbc")
            nc.tensor.matmul(ps_r[:], ones1[:], rrow[:], start=True, stop=True,
                             skip_group_check=True)
            recipf = constp.tile([128, SLICE], BF16)
            nc.vector.tensor_copy(recipf[:], ps_r[:])

            # phase P: ps_P[b] += xT[pair k, b]^T @ AT[pair k] (fp8 DoubleRow)
            ps_Ps = [ps_P_p.tile([128, SLICE], F32, tag=f"pp{bi}",
                                 name=f"ps_P{bi}") for bi in range(B)]

            def pmm(k, bi):
                nc.tensor.matmul(
                    ps_Ps[bi][:],
                    xt[:, k * B + bi, :, :],
                    at[:, k, :, :],
                    start=(k == 0), stop=(k == KP - 1),
                    perf_mode=DR, skip_group_check=True,
                )

            for k in range(KP):
                for bi in range(B):
                    pmm(k, bi)
            # epilogue, batch-pipelined: vector scales P while the PE runs
            # the next batch's passes; scalar copies results out
            for bi in range(B):
                psb = psbp.tile([128, SLICE], BF16, tag="psb", name=f"psb{bi}")
                nc.vector.tensor_mul(psb[:], ps_Ps[bi][:], recipf[:])
                ps_y = ps_y_p.tile([128, SLICE], F32, tag="py",
                                   name=f"ps_y{bi}")
                nc.tensor.matmul(ps_y[:], w1t, psb[:],
                                 start=True, stop=False, skip_group_check=True)
                nc.tensor.matmul(ps_y[:], w0t,
                                 xs[:, bi * SLICE:(bi + 1) * SLICE],
                                 start=False, stop=False, skip_group_check=True)
                nc.tensor.matmul(ps_y[:], blhs[:], brhs[:],
                                 start=False, stop=True, skip_group_check=True)
                ost = ostp.tile([O, SLICE], BF16, tag="ost", name=f"ost{bi}")
                nc.scalar.copy(ost[:], ps_y[:])
                nc.scalar.dma_start(youts[bi][:], ost[:])

    nc.compile()
    return nc


def kernel(x, W, b, idx_i, idx_j):
    in_maps = prep_inputs(x, W, b, idx_i, idx_j)
    nc = build_program()
    res = run_bass_kernel_spmd(nc, in_maps, list(range(NCORES)))
    y = np.empty((B, O, N), np.float32)
    for s in range(NCORES):
        for bi in range(B):
            y[bi, :, s * SLICE:(s + 1) * SLICE] = \
                res.results[s][f"y{bi}"].astype(np.float32)
    return y


if __name__ == "__main__":
    rng = np.random.default_rng(0)
    x = rng.standard_normal((B, C, N), np.float32)
    W = rng.standard_normal((127, C, 2), np.float32) * 0.05
    b = rng.standard_normal((127,), np.float32) * 0.05
    idx_i = rng.integers(0, N, 131072)
    idx_j = rng.integers(0, N, 131072)
    y = kernel(x, W, b, idx_i, idx_j)
    print("ok", y.shape, float(np.abs(y).mean()))
